# revision 1
# baseline (speedup 1.0000x reference)
"""GRU action encoder on 8 trn2 NeuronCores — bf16 matmul version.

Strategy (same structure as baseline, all matmul streams in bf16):
  - Data-parallel: batch N=256 sharded 8 ways (32 per core), weights replicated.
  - Phase 1 (time-parallel precompute): gi = W_ih @ x_t^T (+ b_ih + b_hh[r,z])
    and c1 = W_enc_a @ a_t^T + b_enc for all t -> DRAM (bf16), layout [t][p][cc][n].
  - Phase 2 (sequential scan): per step two matmuls (enc: 1024x1024, hh: 1024x3072)
    streamed as bf16 moving operands through 4 concurrent column-tiled PE groups;
    batch-major partial sums merged + transposed back to feature-major via matmul
    against a replicated R4 (bf16 stationary -> FWL).
  - Gates in fp32 on DVE/ACT, feature-major [128 partitions, 8*32 free].
    Hidden state h kept as bf16 (matmul stationary); gates accumulate fp32.
"""

import sys

sys.path.insert(0, "/opt/trn_rl_repo")

from contextlib import ExitStack

import numpy as np
import ml_dtypes

import concourse.bacc as bacc
import concourse.mybir as mybir
import concourse.tile as tile
from concourse.bass import ds
from concourse.bass_utils import run_bass_kernel_spmd
from concourse.masks import make_identity

N, T, H, A = 256, 128, 1024, 16
NCORES = 8
NL = N // NCORES  # 32 batch per core
HC = H // 128  # 8 feature chunks
F32 = mybir.dt.float32
BF16 = mybir.dt.bfloat16
AF = mybir.ActivationFunctionType
OP = mybir.AluOpType
BF = ml_dtypes.bfloat16


def build_program(repeat: int = 1, static_loop: bool = False, n_steps: int = T):
    nc = bacc.Bacc("TRN2", target_bir_lowering=False, debug=False, num_devices=NCORES)

    xT = nc.declare_dram_parameter("xT", [H, T, NL], BF16, isOutput=False)
    aT = nc.declare_dram_parameter("aT", [A, T, NL], BF16, isOutput=False)
    WencT = nc.declare_dram_parameter("WencT", [H, H], BF16, isOutput=False)
    WencaT = nc.declare_dram_parameter("WencaT", [A, H], BF16, isOutput=False)
    WihT = nc.declare_dram_parameter("WihT", [H, 3 * H], BF16, isOutput=False)
    WhhT = nc.declare_dram_parameter("WhhT", [H, 3 * H], BF16, isOutput=False)
    bias_pre = nc.declare_dram_parameter("bias_pre", [3 * H], F32, isOutput=False)
    b_enc = nc.declare_dram_parameter("b_enc", [H], F32, isOutput=False)
    bhh_n = nc.declare_dram_parameter("bhh_n", [H], BF16, isOutput=False)
    R4 = nc.declare_dram_parameter("R4", [128, NL], BF16, isOutput=False)
    h_out = nc.declare_dram_parameter("h_out", [NL, H], F32, isOutput=True)

    # precomputed per-step data: cc 0..23 = gi (r,z,n chunks), cc 24..31 = c1
    pre_d = nc.dram_tensor("pre_d", [T + 32, 128, 32, NL], BF16)
    pre_ap = pre_d.ap().rearrange("t p c n -> p t c n")

    TB = 8  # timesteps per precompute block
    NB = T // TB  # 16 blocks
    COLS = TB * NL  # 256 columns per precompute matmul

    with tile.TileContext(nc) as tc:
        with ExitStack() as ctx:
            constp = ctx.enter_context(tc.tile_pool(name="const", bufs=1))
            r4_sb = constp.tile([128, NL], BF16)
            nc.sync.dma_start(r4_sb[:], R4[:])
            ident = constp.tile([128, 128], BF16)
            make_identity(nc, ident[:])

            biaspre_sb = constp.tile([128, 24], F32)
            nc.sync.dma_start(biaspre_sb[:], bias_pre.ap().rearrange("(m p) -> p m", p=128))
            benc_sb = constp.tile([128, HC], F32)
            nc.sync.dma_start(benc_sb[:], b_enc.ap().rearrange("(c p) -> p c", p=128))
            bhhn_sb = constp.tile([1, H], BF16)
            nc.sync.dma_start(bhhn_sb[:], bhh_n.ap()[None, :])
            ones_sb = constp.tile([1, NL], BF16)
            nc.gpsimd.memset(ones_sb[:], 1.0)
            wih_sb = constp.tile([128, HC, 3 * H], BF16)
            nc.sync.dma_start(wih_sb[:], WihT.ap().rearrange("(c p) o -> p c o", p=128))
            wenca_sb = constp.tile([A, H], BF16)
            nc.sync.dma_start(wenca_sb[:], WencaT[:])
            wenc_sb = constp.tile([128, HC, H], BF16)
            nc.sync.dma_start(wenc_sb[:], WencT.ap().rearrange("(c p) o -> p c o", p=128))
            whh_sb = constp.tile([128, HC, 3 * H], BF16)
            nc.sync.dma_start(whh_sb[:], WhhT.ap().rearrange("(c p) o -> p c o", p=128))

            if repeat > 1:
                rep_cm = tc.For_i(0, repeat, 1)
                rep_cm.__enter__()

            # ---------------- Phase 1: precompute ----------------
            with ExitStack() as pctx:
                pwork = pctx.enter_context(tc.tile_pool(name="pre_work", bufs=2))
                pps = pctx.enter_context(tc.tile_pool(name="pre_psum", bufs=3, space="PSUM"))

                xT_r = xT.ap().rearrange("(c p) t n -> p c t n", p=128)
                for tb in range(NB):
                    t0 = tb * TB
                    kx = pwork.tile([128, HC, TB, NL], BF16, tag="kx")
                    nc.sync.dma_start(kx[:], xT_r[:, :, t0 : t0 + TB, :])
                    ka = pwork.tile([A, TB, NL], BF16, tag="ka")
                    nc.sync.dma_start(ka[:], aT.ap()[:, t0 : t0 + TB, :])
                    staging = pwork.tile([128, TB, 32, NL], BF16, tag="staging")
                    for m in range(24):
                        pm = pps.tile([128, TB, NL], F32, tag="pm")
                        for c in range(HC):
                            nc.tensor.matmul(
                                pm[:],
                                wih_sb[:, c, 128 * m : 128 * (m + 1)],
                                kx[:, c],
                                start=(c == 0),
                                stop=(c == HC - 1),
                            )
                        nc.scalar.activation(
                            staging[:, :, m, :], pm[:], AF.Identity,
                            bias=biaspre_sb[:, m : m + 1],
                        )
                    for m8 in range(HC):
                        pc = pps.tile([128, TB, NL], F32, tag="pm")
                        nc.tensor.matmul(
                            pc[:],
                            wenca_sb[:, 128 * m8 : 128 * (m8 + 1)],
                            ka[:],
                            start=True,
                            stop=True,
                        )
                        nc.scalar.activation(
                            staging[:, :, 24 + m8, :], pc[:], AF.Identity,
                            bias=benc_sb[:, m8 : m8 + 1],
                        )
                    nc.sync.dma_start(pre_ap[:, t0 : t0 + TB, :, :], staging[:])

            # ---------------- Phase 2: recurrent scan ----------------
            with ExitStack() as lctx:
                state = lctx.enter_context(tc.tile_pool(name="state", bufs=1))
                h_sb = state.tile([128, HC, NL], BF16)
                nc.gpsimd.memset(h_sb[:], 0.0)
                U = 16
                preA = state.tile([128, U, 32, NL], BF16)
                preB = state.tile([128, U, 32, NL], BF16)
                nc.sync.dma_start(preA[:], pre_ap[:, 0:U, :, :])

                work = lctx.enter_context(tc.tile_pool(name="work", bufs=1))
                lps = lctx.enter_context(tc.tile_pool(name="loop_psum", bufs=1, space="PSUM"))

                def step(pre):
                    pre3 = pre[:, 0]  # [128, 32, NL] slab for this t
                    # --- enc matmuls: 4 column groups, group j handles K-chunks {j, j+4}
                    encP = lps.tile([128, 2, 512], F32, tag="mm")
                    for ci in range(2):
                        for j in range(4):
                            c = j + 4 * ci
                            for s in range(2):
                                nc.tensor.matmul(
                                    encP[32 * j : 32 * (j + 1), s, :],
                                    h_sb[:, c, :],
                                    wenc_sb[:, c, 512 * s : 512 * (s + 1)],
                                    start=(ci == 0),
                                    stop=(ci == 1),
                                    tile_position=(0, 32 * j),
                                )
                    # evacuate partials (batch-major) to SBUF as bf16, split DVE/ACT
                    psb = work.tile([128, H], BF16, tag="psb")
                    nc.vector.tensor_copy(psb[:, 0:256], encP[:, 0, 0:256])
                    nc.scalar.copy(psb[:, 256:512], encP[:, 0, 256:512])
                    nc.vector.tensor_copy(psb[:, 512:768], encP[:, 1, 0:256])
                    nc.scalar.copy(psb[:, 768:1024], encP[:, 1, 256:512])
                    # merge 4 col groups + transpose back to feature-major
                    hencP = lps.tile([128, HC, NL], F32, tag="mg")
                    for f in range(HC):
                        nc.tensor.matmul(
                            hencP[:, f, :],
                            psb[:, 128 * f : 128 * (f + 1)],
                            r4_sb[:],
                            start=True,
                            stop=True,
                        )
                    henc = work.tile([128, HC, NL], BF16, tag="henc")
                    nc.vector.tensor_add(henc[:], hencP[:], pre3[:, 24:32, :])

                    # --- hh matmuls ---
                    Q = lps.tile([128, 6, 512], F32, tag="mm")
                    for ci in range(2):
                        for j in range(4):
                            c = j + 4 * ci
                            for s in range(6):
                                st = ci == 0
                                nc.tensor.matmul(
                                    Q[32 * j : 32 * (j + 1), s, :],
                                    henc[:, c, :],
                                    whh_sb[:, c, 512 * s : 512 * (s + 1)],
                                    start=st,
                                    stop=(ci == 1),
                                    tile_position=(0, 32 * j),
                                )
                    qsb = work.tile([128, 3 * H], BF16, tag="qsb")
                    for s in range(6):
                        eng = nc.vector.tensor_copy if s % 2 == 0 else nc.scalar.copy
                        eng(qsb[:, 512 * s : 512 * (s + 1)], Q[:, s, :])
                    ghP = lps.tile([128, 24, NL], F32, tag="mg")
                    for m in range(24):
                        if m >= 16:
                            # seed n-chunks with b_hh_n broadcast over batch
                            nc.tensor.matmul(
                                ghP[:, m, :],
                                bhhn_sb[:, 128 * (m - 16) : 128 * (m - 15)],
                                ones_sb[:],
                                start=True,
                                stop=False,
                            )
                        nc.tensor.matmul(
                            ghP[:, m, :],
                            qsb[:, 128 * m : 128 * (m + 1)],
                            r4_sb[:],
                            start=(m < 16),
                            stop=True,
                        )

                    # --- gates (feature-major, fp32 accumulation) ---
                    rp = work.tile([128, HC, NL], F32, tag="rp")
                    nc.vector.tensor_add(rp[:], ghP[:, 0:8, :], pre3[:, 0:8, :])
                    r = work.tile([128, HC, NL], F32, tag="r")
                    nc.scalar.activation(r[:], rp[:], AF.Sigmoid)
                    zp = work.tile([128, HC, NL], F32, tag="zp")
                    nc.vector.tensor_add(zp[:], ghP[:, 8:16, :], pre3[:, 8:16, :])
                    z = work.tile([128, HC, NL], F32, tag="z")
                    nc.scalar.activation(z[:], zp[:], AF.Sigmoid)
                    # off the n-critical-path: 1-z and z*henc
                    omz = work.tile([128, HC, NL], F32, tag="omz")
                    nc.vector.tensor_scalar(omz[:], z[:], -1.0, 1.0, OP.mult, OP.add)
                    q = work.tile([128, HC, NL], F32, tag="q")
                    nc.vector.tensor_mul(q[:], z[:], henc[:])
                    nm = work.tile([128, HC, NL], F32, tag="nm")
                    nc.vector.tensor_mul(nm[:], r[:], ghP[:, 16:24, :])
                    npre = work.tile([128, HC, NL], F32, tag="npre")
                    nc.vector.tensor_add(npre[:], nm[:], pre3[:, 16:24, :])
                    ngate = work.tile([128, HC, NL], F32, tag="ngate")
                    nc.scalar.activation(ngate[:], npre[:], AF.Tanh)
                    t1 = work.tile([128, HC, NL], F32, tag="t1")
                    nc.vector.tensor_mul(t1[:], ngate[:], omz[:])
                    nc.vector.tensor_add(h_sb[:], t1[:], q[:])

                if static_loop:
                    for t in range(n_steps):
                        if t % U == 0:
                            buf = preB if (t // U) % 2 == 0 else preA
                            nc.sync.dma_start(buf[:], pre_ap[:, t + U : t + 2 * U, :, :])
                        buf = preA if (t // U) % 2 == 0 else preB
                        step(buf[:, t % U : t % U + 1])
                else:
                    with tc.For_i(0, T, 2 * U, hint_engines=(mybir.EngineType.PE,)) as iv:
                        nc.sync.dma_start(preB[:], pre_ap[:, ds(iv + U, U), :, :])
                        for u in range(U):
                            step(preA[:, u : u + 1])
                        nc.sync.dma_start(preA[:], pre_ap[:, ds(iv + 2 * U, U), :, :])
                        for u in range(U):
                            step(preB[:, u : u + 1])

                # ---------------- Phase 3: output transpose ----------------
                outP = lps.tile([32, HC, 128], BF16, tag="mm")
                for c in range(HC):
                    nc.tensor.transpose(outP[:, c, :], h_sb[:, c, :], ident[:])
                hout = work.tile([NL, HC, 128], F32, tag="hout")
                nc.vector.tensor_copy(hout[:], outP[:])
                nc.sync.dma_start(h_out.ap().rearrange("n (c o) -> n c o", c=HC), hout[:])

            if repeat > 1:
                rep_cm.__exit__(None, None, None)

    nc.compile()
    return nc


_cache = {}


def _get_program(repeat: int = 1):
    if repeat not in _cache:
        _cache[repeat] = build_program(repeat)
    return _cache[repeat]


def _prep_inputs(embedding, actions, W_enc, b_enc, W_ih, W_hh, b_ih, b_hh):
    f = np.float32
    xT = np.ascontiguousarray(np.asarray(embedding, f).transpose(2, 1, 0)).astype(BF)
    aT = np.ascontiguousarray(np.asarray(actions, f).transpose(2, 1, 0)).astype(BF)
    W_enc = np.asarray(W_enc, f)
    WencT = np.ascontiguousarray(W_enc[:, :H].T).astype(BF)
    WencaT = np.ascontiguousarray(W_enc[:, H:].T).astype(BF)
    WihT = np.ascontiguousarray(np.asarray(W_ih, f).T).astype(BF)
    WhhT = np.ascontiguousarray(np.asarray(W_hh, f).T).astype(BF)
    b_ih = np.asarray(b_ih, f)
    b_hh = np.asarray(b_hh, f)
    bias_pre = b_ih + np.concatenate([b_hh[: 2 * H], np.zeros(H, f)])
    R4 = np.zeros((128, NL), f)
    R4[np.arange(128), np.arange(128) % NL] = 1.0
    common = dict(
        WencT=WencT, WencaT=WencaT, WihT=WihT, WhhT=WhhT,
        bias_pre=bias_pre, b_enc=np.asarray(b_enc, f),
        bhh_n=b_hh[2 * H :].astype(BF),
        R4=R4.astype(BF),
    )
    in_maps = []
    for k in range(NCORES):
        m = dict(common)
        m["xT"] = np.ascontiguousarray(xT[:, :, k * NL : (k + 1) * NL])
        m["aT"] = np.ascontiguousarray(aT[:, :, k * NL : (k + 1) * NL])
        in_maps.append(m)
    return in_maps


def run(inputs: dict, repeat: int = 1):
    nc = _get_program(repeat)
    in_maps = _prep_inputs(**inputs)
    res = run_bass_kernel_spmd(nc, in_maps, list(range(NCORES)))
    out = np.concatenate([res.results[k]["h_out"] for k in range(NCORES)], axis=0)
    return out


def kernel(**inputs) -> np.ndarray:
    return run(inputs, repeat=1)



# revision 3
# speedup vs baseline: 3.1515x; 3.1515x over previous
"""GRU action encoder on 8 trn2 NeuronCores — fused scan + JIT precompute.

Strategy:
  - Data-parallel: batch N=256 sharded 8 ways (32 per core), weights replicated.
  - Single fused phase: the sequential scan runs step-by-step; the
    time-parallel precompute (gi = W_ih@x + biases, c1 = Wenc_a@a + b_enc)
    for steps t+16..t+31 is interleaved INTO the scan as PE filler work,
    executing in the PE bubbles while DVE/ACT compute the gates.  This keeps
    the PE dense (HAM stays warm at 2.4 GHz) and `pre` lives entirely in
    SBUF (no DRAM round trip).
  - Per step: enc matmul (1024x1024) and hh matmul (1024x3072) streamed as
    bf16 through 4 concurrent column-tiled PE groups (stationary = hidden
    state, moving = weights); batch-major partials merged + transposed back
    to feature-major via matmul against a replicated R4.
  - n-gate bias seeded with a single indicator-matrix matmul into PSUM
    (instead of 8 rank-1 matmuls).
  - Gates in fp32 on DVE/ACT, feature-major [128 partitions, 8*32 free],
    r-gate computed early (off the critical path).
  - pre ring: 4 SBUF buffers of 8 steps each; octave k's filler writes
    buffer (k+2)%4 (WAR-safe, lookahead 16 steps).  kx/ka DMA double-buffered
    with 2-octave lead.
"""

import sys

sys.path.insert(0, "/opt/trn_rl_repo")

from contextlib import ExitStack

import numpy as np
import ml_dtypes

import concourse.bacc as bacc
import concourse.mybir as mybir
import concourse.tile as tile
from concourse.bass import ds
from concourse.bass_utils import run_bass_kernel_spmd
from concourse.masks import make_identity

N, T, H, A = 256, 128, 1024, 16
NCORES = 8
NL = N // NCORES  # 32 batch per core
HC = H // 128  # 8 feature chunks
TB = 8  # timesteps per precompute block (one octave)
NOCT = T // TB  # 16 octaves
TP = T + 32  # padded time (filler/DMA run up to 2 blocks past the end)
F32 = mybir.dt.float32
BF16 = mybir.dt.bfloat16
AF = mybir.ActivationFunctionType
OP = mybir.AluOpType
BF = ml_dtypes.bfloat16


def build_program(repeat: int = 1):
    nc = bacc.Bacc("TRN2", target_bir_lowering=False, debug=False, num_devices=NCORES)

    xT = nc.declare_dram_parameter("xT", [H, TP, NL], BF16, isOutput=False)
    aT = nc.declare_dram_parameter("aT", [A, TP, NL], BF16, isOutput=False)
    WencT = nc.declare_dram_parameter("WencT", [H, H], BF16, isOutput=False)
    WencaT = nc.declare_dram_parameter("WencaT", [A, H], BF16, isOutput=False)
    WihT = nc.declare_dram_parameter("WihT", [H, 3 * H], BF16, isOutput=False)
    WhhT = nc.declare_dram_parameter("WhhT", [H, 3 * H], BF16, isOutput=False)
    bias_pre = nc.declare_dram_parameter("bias_pre", [3 * H], F32, isOutput=False)
    b_enc = nc.declare_dram_parameter("b_enc", [H], F32, isOutput=False)
    bhh8 = nc.declare_dram_parameter("bhh8", [8, 128], BF16, isOutput=False)
    seedR = nc.declare_dram_parameter("seedR", [8, 8 * NL], BF16, isOutput=False)
    R4 = nc.declare_dram_parameter("R4", [128, NL], BF16, isOutput=False)
    h_out = nc.declare_dram_parameter("h_out", [NL, H], F32, isOutput=True)

    xT_r = xT.ap().rearrange("(c p) t n -> p c t n", p=128)
    aT_ap = aT.ap()

    with tile.TileContext(nc) as tc:
        with ExitStack() as ctx:
            constp = ctx.enter_context(tc.tile_pool(name="const", bufs=1))
            r4_sb = constp.tile([128, NL], BF16)
            nc.sync.dma_start(r4_sb[:], R4[:])
            ident = constp.tile([128, 128], BF16)
            make_identity(nc, ident[:])
            biaspre_sb = constp.tile([128, 24], F32)
            nc.sync.dma_start(biaspre_sb[:], bias_pre.ap().rearrange("(m p) -> p m", p=128))
            benc_sb = constp.tile([128, HC], F32)
            nc.sync.dma_start(benc_sb[:], b_enc.ap().rearrange("(c p) -> p c", p=128))
            bhh8_sb = constp.tile([8, 128], BF16)
            nc.sync.dma_start(bhh8_sb[:], bhh8[:])
            seedR_sb = constp.tile([8, 8 * NL], BF16)
            nc.sync.dma_start(seedR_sb[:], seedR[:])
            wih_sb = constp.tile([128, HC, 3 * H], BF16)
            nc.sync.dma_start(wih_sb[:], WihT.ap().rearrange("(c p) o -> p c o", p=128))
            wenca_sb = constp.tile([A, H], BF16)
            nc.sync.dma_start(wenca_sb[:], WencaT[:])
            wenc_sb = constp.tile([128, HC, H], BF16)
            nc.sync.dma_start(wenc_sb[:], WencT.ap().rearrange("(c p) o -> p c o", p=128))
            whh_sb = constp.tile([128, HC, 3 * H], BF16)
            nc.sync.dma_start(whh_sb[:], WhhT.ap().rearrange("(c p) o -> p c o", p=128))

            if repeat > 1:
                rep_cm = tc.For_i(0, repeat, 1)
                rep_cm.__enter__()

            with ExitStack() as lctx:
                state = lctx.enter_context(tc.tile_pool(name="state", bufs=1))
                h_sb = state.tile([128, HC, NL], BF16)
                pre0 = state.tile([128, TB, 32, NL], BF16)
                pre1 = state.tile([128, TB, 32, NL], BF16)
                pre2 = state.tile([128, TB, 32, NL], BF16)
                pre3b = state.tile([128, TB, 32, NL], BF16)
                pres = [pre0, pre1, pre2, pre3b]
                kxF0 = state.tile([128, HC, TB, NL], BF16, tag="kx0")
                kxF1 = state.tile([128, HC, TB, NL], BF16, tag="kx1")
                kxF = [kxF0, kxF1]
                kaF0 = state.tile([A, TB, NL], BF16)
                kaF1 = state.tile([A, TB, NL], BF16)
                kaF = [kaF0, kaF1]

                work = lctx.enter_context(tc.tile_pool(name="work", bufs=1))
                mmp = lctx.enter_context(tc.tile_pool(name="mmp", bufs=3, space="PSUM"))
                hpp = lctx.enter_context(tc.tile_pool(name="hpp", bufs=1, space="PSUM"))
                ghp = lctx.enter_context(tc.tile_pool(name="ghp", bufs=1, space="PSUM"))
                pps = lctx.enter_context(tc.tile_pool(name="pps", bufs=2, space="PSUM"))

                nc.gpsimd.memset(h_sb[:], 0.0)

                def gi_quantum(kx, dst, m):
                    # one output chunk (128 features) of W_ih @ x for 8 steps
                    pm = pps.tile([128, TB, NL], F32, tag="pm")
                    for c in range(HC):
                        nc.tensor.matmul(
                            pm[:],
                            wih_sb[:, c, 128 * m : 128 * (m + 1)],
                            kx[:, c],
                            start=(c == 0),
                            stop=(c == HC - 1),
                        )
                    nc.scalar.activation(
                        dst[:, :, m, :], pm[:], AF.Identity,
                        bias=biaspre_sb[:, m : m + 1],
                    )

                def c1_quantum(ka, dst, f):
                    # one output chunk of Wenc_a @ a + b_enc for 8 steps
                    pm = pps.tile([128, TB, NL], F32, tag="pm")
                    nc.tensor.matmul(
                        pm[:],
                        wenca_sb[:, 128 * f : 128 * (f + 1)],
                        ka[:],
                        start=True,
                        stop=True,
                    )
                    nc.scalar.activation(
                        dst[:, :, 24 + f, :], pm[:], AF.Identity,
                        bias=benc_sb[:, f : f + 1],
                    )

                def filler_quanta(kx, ka, dst):
                    # per-step slices of one 8-step precompute block: 4 per step
                    out = []
                    for u in range(TB):
                        def q(u=u):
                            for i in range(3):
                                gi_quantum(kx, dst, 3 * u + i)
                            c1_quantum(ka, dst, u)
                        out.append(q)
                    return out

                def step(pre3, filler=None):
                    # ---- enc stream: out = Wenc_h @ h (batch-major partials)
                    encS = []
                    for s in range(2):
                        S = mmp.tile([128, 512], F32, tag="mm")
                        for ci in range(2):
                            for j in range(4):
                                c = j + 4 * ci
                                nc.tensor.matmul(
                                    S[32 * j : 32 * (j + 1), :],
                                    h_sb[:, c, :],
                                    wenc_sb[:, c, 512 * s : 512 * (s + 1)],
                                    start=(ci == 0),
                                    stop=(ci == 1),
                                    tile_position=(0, 32 * j),
                                )
                        encS.append(S)
                    psb0 = work.tile([128, 512], BF16, tag="psb0", bufs=2)
                    nc.vector.tensor_copy(psb0[:], encS[0][:])
                    psb1 = work.tile([128, 512], BF16, tag="psb1", bufs=2)
                    nc.scalar.copy(psb1[:], encS[1][:])
                    # ---- merge 4 col groups + transpose to feature-major
                    hencP = hpp.tile([128, HC, NL], F32, tag="hp")
                    for f in range(HC):
                        src = psb0 if f < 4 else psb1
                        nc.tensor.matmul(
                            hencP[:, f, :],
                            src[:, 128 * (f % 4) : 128 * (f % 4 + 1)],
                            r4_sb[:],
                            start=True,
                            stop=True,
                        )
                    # ---- seed n-gate bias into ghP bank B (one indicator matmul)
                    ghP = ghp.tile([128, 24, NL], F32, tag="gh")
                    nc.tensor.matmul(
                        ghP[:, 16:24, :], bhh8_sb[:], seedR_sb[:],
                        start=True, stop=False, skip_group_check=True,
                    )
                    henc = work.tile([128, HC, NL], BF16, tag="henc", bufs=2)
                    nc.vector.tensor_add(henc[:], hencP[:], pre3[:, 24:32, :])

                    # ---- hh stream: 6 col groups of 512, merges one behind
                    qs = []

                    def hh_stream(s):
                        S = mmp.tile([128, 512], F32, tag="mm")
                        for ci in range(2):
                            for j in range(4):
                                c = j + 4 * ci
                                nc.tensor.matmul(
                                    S[32 * j : 32 * (j + 1), :],
                                    henc[:, c, :],
                                    whh_sb[:, c, 512 * s : 512 * (s + 1)],
                                    start=(ci == 0),
                                    stop=(ci == 1),
                                    tile_position=(0, 32 * j),
                                )
                        q = work.tile([128, 512], BF16, tag="qsb", bufs=3)
                        if s % 2 == 0:
                            nc.vector.tensor_copy(q[:], S[:])
                        else:
                            nc.scalar.copy(q[:], S[:])
                        qs.append(q)

                    def hh_merge(s):
                        for k in range(4):
                            m = 4 * s + k
                            nc.tensor.matmul(
                                ghP[:, m, :],
                                qs[s][:, 128 * k : 128 * (k + 1)],
                                r4_sb[:],
                                start=(m < 16),
                                stop=True,
                                skip_group_check=(m >= 16),
                            )

                    hh_stream(0)
                    hh_stream(1)
                    hh_merge(0)
                    hh_stream(2)
                    hh_merge(1)
                    hh_stream(3)
                    # r gate early (off the n critical path)
                    rp = work.tile([128, HC, NL], F32, tag="gadd")
                    nc.vector.tensor_add(rp[:], ghP[:, 0:8, :], pre3[:, 0:8, :])
                    r = work.tile([128, HC, NL], F32, tag="r")
                    nc.scalar.activation(r[:], rp[:], AF.Sigmoid)
                    hh_merge(2)
                    hh_merge(3)
                    hh_stream(4)
                    zp = work.tile([128, HC, NL], F32, tag="gadd")
                    nc.vector.tensor_add(zp[:], ghP[:, 8:16, :], pre3[:, 8:16, :])
                    z = work.tile([128, HC, NL], F32, tag="z")
                    nc.scalar.activation(z[:], zp[:], AF.Sigmoid)
                    hh_stream(5)
                    hh_merge(4)
                    hh_merge(5)

                    # ---- gates (feature-major, fp32)
                    omz = work.tile([128, HC, NL], F32, tag="omz")
                    nc.vector.tensor_scalar(omz[:], z[:], -1.0, 1.0, OP.mult, OP.add)
                    qh = work.tile([128, HC, NL], F32, tag="qh")
                    nc.vector.tensor_mul(qh[:], z[:], henc[:])
                    nm = work.tile([128, HC, NL], F32, tag="nm")
                    nc.vector.tensor_mul(nm[:], r[:], ghP[:, 16:24, :])
                    npre = work.tile([128, HC, NL], F32, tag="gadd")
                    nc.vector.tensor_add(npre[:], nm[:], pre3[:, 16:24, :])
                    ngate = work.tile([128, HC, NL], F32, tag="ngate")
                    nc.scalar.activation(ngate[:], npre[:], AF.Tanh)
                    t1 = work.tile([128, HC, NL], F32, tag="t1")
                    nc.vector.tensor_mul(t1[:], ngate[:], omz[:])
                    nc.vector.tensor_add(h_sb[:], t1[:], qh[:])

                    if filler is not None:
                        filler()

                # ---------------- prologue: fill pre0/pre1, prime kx DMAs
                nc.sync.dma_start(kxF[0][:], xT_r[:, :, 0:TB, :])
                nc.sync.dma_start(kaF[0][:], aT_ap[:, 0:TB, :])
                nc.sync.dma_start(kxF[1][:], xT_r[:, :, TB : 2 * TB, :])
                nc.sync.dma_start(kaF[1][:], aT_ap[:, TB : 2 * TB, :])
                for q in filler_quanta(kxF[0], kaF[0], pres[0]):
                    q()
                nc.sync.dma_start(kxF[0][:], xT_r[:, :, 2 * TB : 3 * TB, :])
                nc.sync.dma_start(kaF[0][:], aT_ap[:, 2 * TB : 3 * TB, :])
                for q in filler_quanta(kxF[1], kaF[1], pres[1]):
                    q()
                nc.sync.dma_start(kxF[1][:], xT_r[:, :, 3 * TB : 4 * TB, :])
                nc.sync.dma_start(kaF[1][:], aT_ap[:, 3 * TB : 4 * TB, :])

                # ---------------- fused scan loop: 4 octaves per iteration
                with tc.For_i(0, T, 4 * TB, hint_engines=(mybir.EngineType.PE,)) as iv:
                    for k in range(4):
                        quanta = filler_quanta(kxF[k % 2], kaF[k % 2], pres[(k + 2) % 4])
                        for u in range(TB):
                            step(pres[k][:, u], filler=quanta[u])
                        # refill the kx/ka slot just consumed (2-octave lead)
                        t0 = 8 * k + 32
                        nc.sync.dma_start(kxF[k % 2][:], xT_r[:, :, ds(iv + t0, TB), :])
                        nc.sync.dma_start(kaF[k % 2][:], aT_ap[:, ds(iv + t0, TB), :])

                # ---------------- epilogue: output transpose
                outP = ghp.tile([32, HC, 128], BF16, tag="gh")
                for c in range(HC):
                    nc.tensor.transpose(outP[:, c, :], h_sb[:, c, :], ident[:])
                hout = state.tile([NL, HC, 128], F32, tag="kx0")
                nc.vector.tensor_copy(hout[:], outP[:])
                nc.sync.dma_start(h_out.ap().rearrange("n (c o) -> n c o", c=HC), hout[:])

            if repeat > 1:
                rep_cm.__exit__(None, None, None)

    nc.compile()
    return nc


_cache = {}


def _get_program(repeat: int = 1):
    if repeat not in _cache:
        _cache[repeat] = build_program(repeat)
    return _cache[repeat]


def _prep_inputs(embedding, actions, W_enc, b_enc, W_ih, W_hh, b_ih, b_hh):
    f = np.float32
    xTf = np.ascontiguousarray(np.asarray(embedding, f).transpose(2, 1, 0))  # [H,T,N]
    aTf = np.ascontiguousarray(np.asarray(actions, f).transpose(2, 1, 0))  # [A,T,N]
    xTp = np.zeros((H, TP, N), f)
    xTp[:, :T, :] = xTf
    aTp = np.zeros((A, TP, N), f)
    aTp[:, :T, :] = aTf
    xTp = xTp.astype(BF)
    aTp = aTp.astype(BF)
    W_enc = np.asarray(W_enc, f)
    WencT = np.ascontiguousarray(W_enc[:, :H].T).astype(BF)
    WencaT = np.ascontiguousarray(W_enc[:, H:].T).astype(BF)
    WihT = np.ascontiguousarray(np.asarray(W_ih, f).T).astype(BF)
    WhhT = np.ascontiguousarray(np.asarray(W_hh, f).T).astype(BF)
    b_ih = np.asarray(b_ih, f)
    b_hh = np.asarray(b_hh, f)
    bias_pre = b_ih + np.concatenate([b_hh[: 2 * H], np.zeros(H, f)])
    bhh8 = np.ascontiguousarray(b_hh[2 * H :].reshape(8, 128)).astype(BF)
    seedR = np.repeat(np.eye(8, dtype=f), NL, axis=1).astype(BF)
    R4 = np.zeros((128, NL), f)
    R4[np.arange(128), np.arange(128) % NL] = 1.0
    common = dict(
        WencT=WencT, WencaT=WencaT, WihT=WihT, WhhT=WhhT,
        bias_pre=bias_pre, b_enc=np.asarray(b_enc, f),
        bhh8=bhh8, seedR=seedR,
        R4=R4.astype(BF),
    )
    in_maps = []
    for k in range(NCORES):
        m = dict(common)
        m["xT"] = np.ascontiguousarray(xTp[:, :, k * NL : (k + 1) * NL])
        m["aT"] = np.ascontiguousarray(aTp[:, :, k * NL : (k + 1) * NL])
        in_maps.append(m)
    return in_maps


def run(inputs: dict, repeat: int = 1):
    nc = _get_program(repeat)
    in_maps = _prep_inputs(**inputs)
    res = run_bass_kernel_spmd(nc, in_maps, list(range(NCORES)))
    out = np.concatenate([res.results[k]["h_out"] for k in range(NCORES)], axis=0)
    return out


def kernel(**inputs) -> np.ndarray:
    return run(inputs, repeat=1)
